# revision 33
# baseline (speedup 1.0000x reference)
"""CRPS loss kernel for Trainium2, 8 NeuronCores (SPMD data-parallel).

reference semantics:
    p, t = prediction.ravel(), target.ravel()       # N = 16,611,840 each
    lo, hi = min(min p, min t), max(max p, max t)
    x = linspace(lo, hi, 1000)  (f32)
    cdf_q(x_i) = #{v in q : v <= x_i} / N
    return trapz(|cdf_p - cdf_t|^2, x)

Device work (per core, 1/8 shard of each tensor):
  kernel A: running min/max reduce  -> per-core (min, -max)
  kernel B: per element j = rint(v*A + B) (A = 1/dx, B = -lo*A + 0.5) as i32;
            digits a = j>>5, b = j&31. Joint histogram via PE outer products,
            PACK=4 consecutive elements per matmul with CONTIGUOUS operands:
              weights  = 32 a-cols 0/1 thermometer is_ge(j, 32*qa - 0.5)
                         built at DVE 4x from j held exactly in f16
              moving   = 32 b-cols: low cols eq(b, qb) on DVE; high ACT_Q
                         cols Sign(b - qb - 0.5) thermometer (+-1) on ScalarE
            Column layout col = E*128 + q*4 + g (E = matmul idx, g = elem&3)
            keeps one-hot writes in unit-stride 4-runs (DVE stays 4x) while
            matmul E reads one contiguous [128, 128] slab (FWL + fast
            stream). PSUM [128, 128] accumulates a whole tensor; 2 drains.
Host: decode thermometer columns (cumulative -> counts), fold j>=999,
      cumsum, 1000-point trapz in f64.

Shards are padded to [128, 16384] with a copy of the shard's first PADN
elements; the host subtracts their exact histogram (same f32 math,
vectorized).
"""

import numpy as np
from concourse import bacc, mybir, tile
from concourse.bass_utils import run_bass_kernel_spmd

P = 128
NCORES = 8
TOTAL = 16 * 1 * 721 * 1440          # 16,611,840
SHARD = TOTAL // NCORES              # 2,076,480
KTOT = 16384                         # padded columns/core/tensor
PADN = P * KTOT - SHARD              # 20,672
NB = 32                              # 32x32 = 1024 bins
NX = 1000
CHUNK = 512
NCHUNK = KTOT // CHUNK               # 32
PACK = 4                             # consecutive elements per matmul
EPC = CHUNK // PACK                  # matmuls per chunk (128)
ACT_Q = 18                           # b-side thermometer cols on ScalarE
RED_CHUNK = 2048
F32 = mybir.dt.float32
F16 = mybir.dt.float16
I32 = mybir.dt.int32
I16 = mybir.dt.int16
BF16 = mybir.dt.bfloat16
ALU = mybir.AluOpType
AF = mybir.ActivationFunctionType


def _build_minmax():
    nc = bacc.Bacc()
    ins = [
        nc.declare_dram_parameter("pv", [P, KTOT], F32, isOutput=False),
        nc.declare_dram_parameter("tv", [P, KTOT], F32, isOutput=False),
    ]
    out = nc.declare_dram_parameter("mm", [1, 2], F32, isOutput=True)  # (-min, max)

    with tile.TileContext(nc) as tc:
        with (
            tc.tile_pool(name="sbuf", bufs=4) as pool,
            tc.tile_pool(name="acc", bufs=1) as apool,
        ):
            nred = (KTOT // RED_CHUNK) * 2
            mins = apool.tile([P, nred], F32)
            maxs = apool.tile([P, nred], F32)
            col = 0
            for src in ins:
                for ci in range(KTOT // RED_CHUNK):
                    v = pool.tile([P, RED_CHUNK], F32, tag="v")
                    nc.sync.dma_start(v[:], src[:, ci * RED_CHUNK:(ci + 1) * RED_CHUNK])
                    nc.vector.tensor_reduce(
                        mins[:, col:col + 1], v[:], mybir.AxisListType.X, ALU.min)
                    nc.vector.tensor_reduce(
                        maxs[:, col:col + 1], v[:], mybir.AxisListType.X, ALU.max)
                    col += 1
            pmin = apool.tile([P, 1], F32)
            pmax = apool.tile([P, 1], F32)
            nc.vector.tensor_reduce(pmin[:], mins[:], mybir.AxisListType.X, ALU.min)
            nc.vector.tensor_reduce(pmax[:], maxs[:], mybir.AxisListType.X, ALU.max)
            # cross-lane reduce only supports add/average/max -> store (-min, max)
            both = apool.tile([P, 2], F32)
            nc.vector.tensor_scalar(out=both[:, 0:1], in0=pmin[:], scalar1=-1.0,
                                    scalar2=None, op0=ALU.mult)
            nc.vector.tensor_copy(out=both[:, 1:2], in_=pmax[:])
            red = apool.tile([1, 2], F32)
            nc.gpsimd.tensor_reduce(red[:], both[:], mybir.AxisListType.C, ALU.max)
            nc.sync.dma_start(out[:], red[:])
    nc.compile()
    return nc


def _build_hist():
    nc = bacc.Bacc()
    ins = [
        nc.declare_dram_parameter("pv", [P, KTOT], F32, isOutput=False),
        nc.declare_dram_parameter("tv", [P, KTOT], F32, isOutput=False),
    ]
    ab_in = nc.declare_dram_parameter("ab", [P, 2], F32, isOutput=False)
    # per tensor a [128, 128] lattice: cell (qa*4+g, qb*4+g) = c(qa, qb)
    # (thermo-coded for high qb)
    out = nc.declare_dram_parameter("hist", [P, 2 * PACK * NB], F32, isOutput=True)

    with tile.TileContext(nc) as tc:
        with (
            tc.tile_pool(name="sbuf", bufs=4) as pool,
            tc.tile_pool(name="oh", bufs=2) as ohpool,
            tc.tile_pool(name="const", bufs=1) as cpool,
            tc.tile_pool(name="acc", bufs=1) as apool,
            tc.tile_pool(name="psum", bufs=2, space="PSUM") as psum_pool,
        ):
            ab_raw = cpool.tile([P, 2], F32)
            nc.sync.dma_start(ab_raw[:], ab_in[:])
            ab = cpool.tile([P, 2], F32)
            nc.vector.tensor_copy(out=ab[:], in_=ab_raw[:])
            # ScalarE Sign bias APs: -(q + 0.5) per thermometer column
            sbias = cpool.tile([P, NB], F32)
            for q in range(NB - ACT_Q, NB):
                nc.vector.memset(sbias[:, q:q + 1], -(float(q) + 0.5))

            hsb = apool.tile([P, 2 * PACK * NB], F32)
            psums = [
                psum_pool.tile([NB * PACK, NB * PACK], F32, space="PSUM",
                               tag=f"m{t}", name=f"m{t}")
                for t in range(2)
            ]

            for ti, src in enumerate(ins):
                m_psum = psums[ti]
                for ci in range(NCHUNK):
                    v = pool.tile([P, CHUNK], F32, tag="v")
                    nc.sync.dma_start(v[:], src[:, ci * CHUNK:(ci + 1) * CHUNK])
                    # u = rint(v*A + B) directly (convert-on-write is RNE)
                    u = pool.tile([P, CHUNK], I32, tag="u")
                    nc.vector.tensor_scalar(out=u[:], in0=v[:],
                                            scalar1=ab[:, 0:1], scalar2=ab[:, 1:2],
                                            op0=ALU.mult, op1=ALU.add)
                    b32 = pool.tile([P, CHUNK], I32, tag="b32")
                    nc.vector.tensor_scalar(out=b32[:], in0=u[:], scalar1=31,
                                            scalar2=None, op0=ALU.bitwise_and)
                    # f16 holds j (<= 1000) exactly; bf16 digits for b
                    uc = pool.tile([P, CHUNK], F16, tag="uc")
                    nc.vector.tensor_copy(out=uc[:], in_=u[:])
                    bic = pool.tile([P, CHUNK], BF16, tag="bic")
                    nc.vector.tensor_copy(out=bic[:], in_=b32[:])

                    # encodings, col = E*W + q*4 + g (g = elem & 3, E = elem >> 2)
                    oh_a = ohpool.tile([P, NB * CHUNK], BF16, tag="oh_a")
                    oh_b = ohpool.tile([P, NB * CHUNK], BF16, tag="oh_b")
                    ohav = oh_a[:].rearrange("p (e q g) -> p q e g",
                                             e=EPC, q=NB, g=PACK)
                    ohbv = oh_b[:].rearrange("p (e q g) -> p q e g",
                                             e=EPC, q=NB, g=PACK)
                    # a-side 0/1 thermometer: 1[j >= 32q] == 1[a >= q]
                    for q in range(NB):
                        nc.vector.tensor_scalar(
                            out=ohav[:, q], in0=uc[:], scalar1=32.0 * q - 0.5,
                            scalar2=None, op0=ALU.is_ge)
                    for q in range(NB):
                        if q == NB - 1:
                            # ones column: matmul emits row totals directly
                            nc.vector.memset(ohbv[:, q], 1.0)
                        elif q < NB - ACT_Q:
                            nc.vector.tensor_scalar(
                                out=ohbv[:, q], in0=bic[:], scalar1=float(q),
                                scalar2=None, op0=ALU.is_equal)
                        else:
                            nc.scalar.activation(
                                ohbv[:, q], bic[:], AF.Sign,
                                bias=sbias[:, q:q + 1], scale=1.0)
                    for e in range(EPC):
                        nc.tensor.matmul(
                            m_psum[:],
                            lhsT=oh_a[:, e * NB * PACK:(e + 1) * NB * PACK],
                            rhs=oh_b[:, e * NB * PACK:(e + 1) * NB * PACK],
                            start=(ci == 0 and e == 0),
                            stop=(ci == NCHUNK - 1 and e == EPC - 1),
                        )
                nc.vector.tensor_copy(
                    out=hsb[:, ti * PACK * NB:(ti + 1) * PACK * NB],
                    in_=m_psum[:])
            nc.sync.dma_start(out[:], hsb[:])
    nc.compile()
    return nc


_KERNELS = {}


def _get_kernels():
    if "mm" not in _KERNELS:
        _KERNELS["mm"] = _build_minmax()
        _KERNELS["hist"] = _build_hist()
    return _KERNELS["mm"], _KERNELS["hist"]


def _shard(flat):
    """Split [TOTAL] -> per-core padded [P, KTOT] tiles + pad arrays."""
    tiles, pads = [], []
    for c in range(NCORES):
        s = flat[c * SHARD:(c + 1) * SHARD]
        pv = s[:PADN]
        t = np.concatenate([s, pv]).reshape(P, KTOT)
        tiles.append(t)
        pads.append(pv)
    return tiles, pads


def _bins_of(v, A, B):
    """Replicate device binning (f32 mult, f32 add, rint) for an array."""
    v = np.asarray(v, np.float32)
    t1 = (v * np.float32(A)).astype(np.float32)
    z = (t1 + np.float32(B)).astype(np.float32)
    j = np.rint(z.astype(np.float64)).astype(np.int64)
    return np.clip(j, 0, NB * NB - 1)


def _decode_lattice(h):
    """[128, 128] psum block -> [1024] histogram (f64, exact)."""
    H = np.zeros((NB, NB), np.float64)
    eqn = NB - ACT_Q
    for g in range(PACK):
        X = h[g::PACK, g::PACK]                    # [NB, NB] counts
        X = np.vstack([X[:-1] - X[1:], X[-1:]])    # undo a-thermometer rows
        Hg = np.empty((NB, NB), np.float64)
        Hg[:, :eqn] = X[:, :eqn]
        if ACT_Q > 0:
            T = X[:, NB - 1]                       # ones col 31 = total
            C = (T[:, None] - X[:, eqn:NB - 1]) / 2.0   # cumulative <= qt
            Hg[:, eqn] = C[:, 0] - Hg[:, :eqn].sum(axis=1)
            Hg[:, eqn + 1:NB - 1] = C[:, 1:] - C[:, :-1]
            Hg[:, NB - 1] = T - C[:, -1]
        H += Hg
    return H.ravel()


def kernel(prediction, target):
    nc_mm, nc_hist = _get_kernels()
    p = np.ascontiguousarray(np.asarray(prediction, dtype=np.float32).ravel())
    t = np.ascontiguousarray(np.asarray(target, dtype=np.float32).ravel())
    p_tiles, p_pads = _shard(p)
    t_tiles, t_pads = _shard(t)
    core_ids = list(range(NCORES))

    in_maps = [{"pv": p_tiles[c], "tv": t_tiles[c]} for c in core_ids]
    res = run_bass_kernel_spmd(nc_mm, in_maps, core_ids).results
    mm = np.stack([r["mm"][0] for r in res])        # [8, 2] = (-min, max)
    lo = np.float32(-(mm[:, 0].max()))
    hi = np.float32(mm[:, 1].max())

    dx = np.float32((hi - lo) / np.float32(NX - 1))
    A = np.float32(np.float32(1.0) / dx)
    B = np.float32(np.float32(-lo * A) + np.float32(0.5))
    ab = np.stack([np.full(P, A, np.float32), np.full(P, B, np.float32)], axis=1)

    in_maps = [{"pv": p_tiles[c], "tv": t_tiles[c], "ab": ab} for c in core_ids]
    res = run_bass_kernel_spmd(nc_hist, in_maps, core_ids).results

    hp = np.zeros(NB * NB, np.float64)
    ht = np.zeros(NB * NB, np.float64)
    W = PACK * NB
    for c in core_ids:
        h = res[c]["hist"].astype(np.float64)      # [P, 2*W]
        hp += _decode_lattice(h[:, :W])
        ht += _decode_lattice(h[:, W:])
        hp -= np.bincount(_bins_of(p_pads[c], A, B), minlength=NB * NB)
        ht -= np.bincount(_bins_of(t_pads[c], A, B), minlength=NB * NB)

    # fold j >= NX-1 into bin NX-1, cumsum -> counts at x_i
    hp[NX - 1] += hp[NX:].sum()
    ht[NX - 1] += ht[NX:].sum()
    cnt_p = np.cumsum(hp[:NX])
    cnt_t = np.cumsum(ht[:NX])

    n = np.float64(TOTAL)
    diff = np.abs(cnt_p / n - cnt_t / n)
    y = diff * diff
    x = np.linspace(np.float64(lo), np.float64(hi), NX)
    dxs = x[1:] - x[:-1]
    out = np.sum(0.5 * (y[1:] + y[:-1]) * dxs)
    return np.float32(out)
